# revision 29
# baseline (speedup 1.0000x reference)
"""Causal multi-head attention (B=2, S=2048, D=1024, H=16, hd=64) on 8 trn2 cores.

Sharding: core = (batch b, head-group g): cores 0-3 -> batch 0, groups 0-3;
cores 4-7 -> batch 1. Each core computes 4 heads of one batch element:
QKV projections for its 256 hd-dims, causal attention, and a partial output
projection (attn_heads @ Wo.T restricted to its hd columns). Host sums the 4
partials per batch and adds bo (the "all-reduce after the output projection").

On-core layout (everything fp32r on the PE, fp32 accumulation in PSUM):
  Q^T, K^T  [hd 128(2 heads/pair), pair 2, tok 2048]   (hd on partitions)
  scores^T  [k_tok 128/block, q 512]  = K^T.T @ Q^T    (per head, per k-block)
  causal mask added in PSUM via identity-matmul with a host-built additive mask
  probs^T = Exp(scale * scores^T)  via ScalarE, PSUM -> SBUF (no max-subtract:
            scores are O(5), exp is safe in fp32; masked entries exp -> 0)
  PV: attn^T[65, q] = V_aug.T @ probs^T, accumulated over k-blocks, where
      V_aug = [V | ones] so row 64 is the softmax denominator.
  normalize: denom rows bounce via DRAM, reciprocal computed wide on [128,8],
  then partition-broadcast back with a stride-0 DRAM read; attn^T *= 1/denom.
  out partial = attn^T.T @ Wo_cols^T  (attn^T is directly the lhsT).
  x and Wq/Wk/Wv stream in as bf16 (halves input DMA); K^T is stored as two
  zero-padded per-head tiles so every scores matmul runs at full K=128.
  Measured: ~222 us HW exec (8 cores), rel err ~2e-3 vs fp32 reference.
"""
import sys

sys.path.insert(0, "/opt/trn_rl_repo")

import numpy as np
import ml_dtypes

import concourse.bass as bass
import concourse.bacc as bacc
import concourse.tile as tile
import concourse.mybir as mybir
from concourse.bass_utils import run_bass_kernel_spmd

B, S, D, H, HD = 2, 2048, 1024, 16, 64
HPC = 4            # heads per core
HDC = HPC * HD     # 256 hd dims per core
KC = D // 128      # 8 contraction chunks
TQ = S // 512      # 4 q-chunks of 512
TT = S // 128      # 16 token tiles of 128
SCALE = 1.0 / 8.0  # 1/sqrt(64)
NEG = -1.0e9

f32 = mybir.dt.float32
f32r = mybir.dt.float32r
bf16 = mybir.dt.bfloat16
PROJ_BF16 = True  # x and Wq/Wk/Wv in bf16 (halves input DMA); attention stays f32r
PROJ_DT = bf16 if PROJ_BF16 else f32r

_CACHE = {}


def _emit(tc, d, ctx):
    nc = tc.nc
    singles = ctx.enter_context(tc.tile_pool(name="singles", bufs=1))
    xt_pool = ctx.enter_context(tc.tile_pool(name="xt", bufs=6))
    probs_pool = ctx.enter_context(tc.tile_pool(name="probs", bufs=4))
    stage_pool = ctx.enter_context(tc.tile_pool(name="stage", bufs=3))
    norm_pool = ctx.enter_context(tc.tile_pool(name="norm", bufs=2))
    ps = ctx.enter_context(tc.tile_pool(name="ps", bufs=4, space="PSUM"))

    bias_sb = singles.tile([128, 6], f32)
    nc.sync.dma_start(out=bias_sb, in_=d["bias"][:])

    # ---- projections: Q^T, K^T, V^T  (K-outer, 8 psum cells = 4 ps2 tiles) ----
    qt_sb = singles.tile([128, 2, S], f32r, tag="qt")
    ktz_sb = singles.tile([128, 2, 2, S], f32r, tag="ktz")
    nc.vector.memset(ktz_sb[64:128, :, 0, :].bitcast(f32), 0.0)
    nc.vector.memset(ktz_sb[0:64, :, 1, :].bitcast(f32), 0.0)
    vtt_sb = singles.tile([128, 2, S], f32r, tag="vtt")
    w_sb = {}
    proj = [("xq", "wq", qt_sb, 0), ("xk", "wk", None, 2), ("xv", "wv", vtt_sb, 4)]
    for xnm, wnm, dst, bcol in proj:
        w_sb[wnm] = singles.tile([128, KC, HDC], PROJ_DT, tag=wnm, name=wnm)
        nc.sync.dma_start(
            out=w_sb[wnm], in_=d[wnm][:].rearrange("p (kc m) -> p kc m", kc=KC)
        )
        cellt = [
            ps.tile([128, 2, 512], f32, tag="ps2", name=f"cell_{xnm}_{i}")
            for i in range(4)
        ]

        def cell(mc, t):
            i = mc * TQ + t
            return cellt[i // 2][:, i % 2, :]

        for c in range(KC):
            xt = xt_pool.tile([128, S], PROJ_DT, tag="xt")
            nc.sync.dma_start(out=xt, in_=d[xnm][c * 128 : (c + 1) * 128, :])
            for mc in range(2):
                for t in range(TQ):
                    nc.tensor.matmul(
                        cell(mc, t),
                        w_sb[wnm][:, c, mc * 128 : (mc + 1) * 128],
                        xt[:, t * 512 : (t + 1) * 512],
                        start=(c == 0),
                        stop=(c == KC - 1),
                    )
        for mc in range(2):
            for t in range(TQ):
                tsl = slice(t * 512, (t + 1) * 512)
                if dst is None:  # K^T: split into zero-padded per-head tiles
                    nc.vector.tensor_scalar_add(
                        out=ktz_sb[0:64, mc, 0, tsl],
                        in0=cell(mc, t)[0:64, :],
                        scalar1=bias_sb[0:64, bcol + mc : bcol + mc + 1],
                    )
                    nc.vector.tensor_scalar_add(
                        out=ktz_sb[64:128, mc, 1, tsl],
                        in0=cell(mc, t)[64:128, :],
                        scalar1=bias_sb[64:128, bcol + mc : bcol + mc + 1],
                    )
                else:
                    nc.vector.tensor_scalar_add(
                        out=dst[:, mc, tsl],
                        in0=cell(mc, t),
                        scalar1=bias_sb[:, bcol + mc : bcol + mc + 1],
                    )

    # ---- V: transpose V^T -> V natural [tok, hd] per head, append ones ----
    ident = singles.tile([128, 128], f32r)
    nc.sync.dma_start(out=ident, in_=d["ident"][:])
    mask_sb = singles.tile([128, 4, 512], f32r)
    nc.sync.dma_start(out=mask_sb, in_=d["mask"][:])
    v_sb = [
        singles.tile([128, TT, 65], f32r, tag=f"v{h}", name=f"v{h}")
        for h in range(HPC)
    ]
    for h in range(HPC):
        nc.vector.memset(v_sb[h][:, :, 64:65].bitcast(f32), 1.0)
    for c in range(2):
        for t in range(TT):
            tp = ps.tile([128, 2, 512], f32r, tag="ps2")
            with nc.allow_low_precision(reason="f32r transpose; psum is fp32"):
                nc.tensor.transpose(
                    tp[:, 0, 0:128], vtt_sb[:, c, t * 128 : (t + 1) * 128], ident
                )
            for h2 in range(2):
                h = 2 * c + h2
                nc.vector.tensor_copy(
                    out=v_sb[h][:, t, 0:64], in_=tp[:, 0, h2 * 64 : (h2 + 1) * 64]
                )

    # ---- attention (per head-pair p, per q-chunk t, k-blocks in pairs) ----
    wo_sb = singles.tile([128, 2, D], f32r)
    nc.sync.dma_start(out=wo_sb, in_=d["wo"][:].rearrange("p (c o) -> p c o", c=2))
    attnt_sb = singles.tile([128, 2, S], f32r, tag="attnt")
    for p in range(2):
        for t in range(TQ):
            q0 = t * 512
            qsl = slice(q0, q0 + 512)
            nkb = q0 // 128 + 4
            pvt = ps.tile([128, 2, 512], f32, tag="ps2", name=f"pv_{p}_{t}")
            for kb0 in range(0, nkb, 2):
                for h2 in range(2):
                    h = 2 * p + h2
                    scg = ps.tile([128, 2, 512], f32, tag="ps2", name="scg")
                    for j in range(2):
                        kb = kb0 + j
                        dg = kb - q0 // 128
                        nc.tensor.matmul(
                            scg[:, j, :],
                            ktz_sb[:, p, h2, kb * 128 : (kb + 1) * 128],
                            qt_sb[:, p, qsl],
                            start=True,
                            stop=(dg < 0),
                        )
                        if dg >= 0:
                            nc.tensor.matmul(
                                scg[:, j, :],
                                ident[:],
                                mask_sb[:, dg, :],
                                start=False,
                                stop=True,
                            )
                    pr = probs_pool.tile([128, 2, 512], f32r, tag="probs")
                    nc.scalar.activation(
                        out=pr[:],
                        in_=scg[:],
                        func=mybir.ActivationFunctionType.Exp,
                        scale=SCALE,
                    )
                    for j in range(2):
                        kb = kb0 + j
                        nc.tensor.matmul(
                            pvt[0:65, h2, :],
                            v_sb[h][:, kb, :],
                            pr[:, j, :],
                            start=(kb == 0),
                            stop=(kb == nkb - 1),
                        )
            dn = norm_pool.tile([65, 2, 512], f32, tag="dn")
            tmpb = norm_pool.tile([64, 512], f32r, tag="tmpb")
            bc_sb = norm_pool.tile([128, 512], f32, tag="bcs")
            # head A: rows 0-63 partition-preserving copy
            nc.vector.tensor_copy(out=attnt_sb[0:64, p, qsl], in_=pvt[0:64, 0, :])
            # head B: bounce through SBUF then DMA to partitions 64-127
            nc.vector.tensor_copy(out=tmpb[:], in_=pvt[0:64, 1, :])
            nc.sync.dma_start(out=attnt_sb[64:128, p, qsl], in_=tmpb[:])
            nc.vector.tensor_copy(out=dn[64:65, 0, :], in_=pvt[64:65, 0, :])
            nc.vector.tensor_copy(out=dn[64:65, 1, :], in_=pvt[64:65, 1, :])
            # denoms -> DRAM; read back wide [128,8]; exact reciprocal on
            # 128 lanes; back to DRAM; stride-0 broadcast read per head
            for h2 in range(2):
                nc.sync.dma_start(out=d["nscr"][p, t, h2, :], in_=dn[64:65, h2, :])
            wide = norm_pool.tile([128, 8], f32, tag="wide")
            wrec = norm_pool.tile([128, 8], f32, tag="wrec")
            flat_in = d["nscr"][p, t].rearrange("c q -> (c q)").rearrange(
                "(pp f) -> pp f", pp=128
            )
            nc.sync.dma_start(out=wide[:], in_=flat_in)
            with nc.allow_low_precision(reason="softmax denominators, fp32"):
                nc.vector.reciprocal(out=wrec[:], in_=wide[:])
            flat_out = d["nscr2"][p, t].rearrange("c q -> (c q)").rearrange(
                "(pp f) -> pp f", pp=128
            )
            nc.sync.dma_start(out=flat_out, in_=wrec[:])
            for h2 in range(2):
                srcd = d["nscr2"][p, t, h2, :]
                rep = bass.AP(
                    tensor=srcd.tensor,
                    offset=srcd.offset,
                    ap=[[0, 64]] + [list(e) for e in srcd.ap],
                )
                nc.sync.dma_start(
                    out=bc_sb[h2 * 64 : (h2 + 1) * 64, :], in_=rep
                )
            nc.vector.tensor_tensor(
                out=attnt_sb[:, p, qsl],
                in0=attnt_sb[:, p, qsl],
                in1=bc_sb[:],
                op=mybir.AluOpType.mult,
            )
            nc.vector.tensor_scalar_add(
                out=attnt_sb[:, p, qsl],
                in0=attnt_sb[:, p, qsl],
                scalar1=bias_sb[:, 4 + p : 5 + p],
            )
            if p == 1:
                # out-projection for this q-chunk's token tiles (both pairs
                # of attn^T are now normalized for these tokens)
                for tt in range(t * 4, t * 4 + 4):
                    tsl = slice(tt * 128, (tt + 1) * 128)
                    po = ps.tile([128, 2, 512], f32, tag="ps2", name=f"po_{tt}")
                    for o in range(2):
                        osl = slice(o * 512, (o + 1) * 512)
                        for c in range(2):
                            nc.tensor.matmul(
                                po[:, o, :],
                                attnt_sb[:, c, tsl],
                                wo_sb[:, c, osl],
                                start=(c == 0),
                                stop=(c == 1),
                            )
                    st = stage_pool.tile([128, 2, 512], f32, tag="st")
                    nc.vector.tensor_copy(out=st[:], in_=po[:])
                    nc.sync.dma_start(
                        out=d["out"][tsl, :],
                        in_=st[:].rearrange("p a b -> p (a b)"),
                    )


def _build_nc():
    nc = bacc.Bacc()
    d = {}
    for nm in ("xq", "xk", "xv"):
        d[nm] = nc.declare_dram_parameter(nm, [D, S], PROJ_DT, isOutput=False)
    for nm in ("wq", "wk", "wv"):
        d[nm] = nc.declare_dram_parameter(nm, [128, KC * HDC], PROJ_DT, isOutput=False)
    d["wo"] = nc.declare_dram_parameter("wo", [128, 2 * D], f32r, isOutput=False)
    d["bias"] = nc.declare_dram_parameter("bias", [128, 6], f32, isOutput=False)
    d["mask"] = nc.declare_dram_parameter("mask", [128, 4, 512], f32r, isOutput=False)
    d["ident"] = nc.declare_dram_parameter("ident", [128, 128], f32r, isOutput=False)
    d["out"] = nc.declare_dram_parameter("out", [S, D], f32, isOutput=True)
    from contextlib import ExitStack

    d["nscr"] = nc.dram_tensor("nscr", [2, TQ, 2, 512], f32)
    d["nscr2"] = nc.dram_tensor("nscr2", [2, TQ, 2, 512], f32)
    with tile.TileContext(nc) as tc:
        with ExitStack() as ctx:
            _emit(tc, d, ctx)
    nc.compile()
    return nc


def _get_nc():
    if "nc" not in _CACHE:
        _CACHE["nc"] = _build_nc()
    return _CACHE["nc"]


_PROJ_NP = ml_dtypes.bfloat16 if PROJ_BF16 else np.float32


def _xarr(xt):
    return np.ascontiguousarray(xt).astype(_PROJ_NP)


def _warr(wt):  # [D, HDC] -> [128, KC*HDC] chunk-contiguous
    return np.ascontiguousarray(
        wt.reshape(KC, 128, HDC).transpose(1, 0, 2).reshape(128, KC * HDC)
    ).astype(_PROJ_NP)


def _woarr(wt):  # [HDC, D] -> [128, 2*D]
    return np.ascontiguousarray(
        wt.reshape(2, 128, D).transpose(1, 0, 2).reshape(128, 2 * D)
    )


def _host_consts():
    mask = np.zeros((128, 4, 512), np.float32)
    p = np.arange(128)[:, None]
    j = np.arange(512)[None, :]
    for dg in range(4):
        mask[:, dg, :] = np.where(128 * dg + p <= j, 0.0, NEG)
    ident = np.eye(128, dtype=np.float32)
    return mask, ident


def kernel(trace=False, **inputs):
    q = np.asarray(inputs["q"], np.float32)
    k = np.asarray(inputs["k"], np.float32)
    v = np.asarray(inputs["v"], np.float32)
    Wq = np.asarray(inputs["Wq"], np.float32)
    Wk = np.asarray(inputs["Wk"], np.float32)
    Wv = np.asarray(inputs["Wv"], np.float32)
    Wo = np.asarray(inputs["Wo"], np.float32)
    bq = np.asarray(inputs["bq"], np.float32)
    bk = np.asarray(inputs["bk"], np.float32)
    bv = np.asarray(inputs["bv"], np.float32)
    bo = np.asarray(inputs["bo"], np.float32)
    # inputs["mask"] is the causal tril mask, baked into the kernel.

    mask, ident = _host_consts()
    nc = _get_nc()
    in_maps = []
    for core in range(8):
        b, g = core // 4, core % 4
        sl = slice(g * HDC, (g + 1) * HDC)
        bias = np.zeros((128, 6), np.float32)
        for col, bvec in ((0, bq), (2, bk), (4, bv)):
            seg = bvec[sl].reshape(2, 128)
            bias[:, col] = seg[0]
            bias[:, col + 1] = seg[1]
        in_maps.append(
            {
                "xq": _xarr(q[b].T),
                "xk": _xarr(k[b].T),
                "xv": _xarr(v[b].T),
                "wq": _warr(Wq[sl, :].T),
                "wk": _warr(Wk[sl, :].T),
                "wv": _warr(Wv[sl, :].T),
                "wo": _woarr(Wo[:, sl].T),
                "bias": bias,
                "mask": mask,
                "ident": ident,
            }
        )
    res = run_bass_kernel_spmd(nc, in_maps, core_ids=list(range(8)), trace=trace)
    outs = [r["out"] for r in res.results]
    final = np.empty((B, S, D), np.float32)
    for b in range(B):
        final[b] = outs[4 * b] + outs[4 * b + 1] + outs[4 * b + 2] + outs[4 * b + 3]
        final[b] += bo
    if trace:
        kernel.last_exec_time_ns = res.exec_time_ns
        kernel.last_results = res
    return final


# revision 30
# speedup vs baseline: 1.2430x; 1.2430x over previous
"""Causal multi-head attention (B=2, S=2048, D=1024, H=16, hd=64) on 8 trn2 cores.

Sharding: core = (batch b, head-group g): cores 0-3 -> batch 0, groups 0-3;
cores 4-7 -> batch 1. Each core computes 4 heads of one batch element:
QKV projections for its 256 hd-dims, causal attention, and a partial output
projection (attn_heads @ Wo.T restricted to its hd columns). Host sums the 4
partials per batch and adds bo (the "all-reduce after the output projection").

On-core layout (everything fp32r on the PE, fp32 accumulation in PSUM):
  Q^T, K^T  [hd 128(2 heads/pair), pair 2, tok 2048]   (hd on partitions)
  scores^T  [k_tok 128/block, q 512]  = K^T.T @ Q^T    (per head, per k-block)
  causal mask added in PSUM via identity-matmul with a host-built additive mask
  probs^T = Exp(scale * scores^T)  via ScalarE, PSUM -> SBUF (no max-subtract:
            scores are O(5), exp is safe in fp32; masked entries exp -> 0)
  PV: attn^T[65, q] = V_aug.T @ probs^T, accumulated over k-blocks, where
      V_aug = [V | ones] so row 64 is the softmax denominator.
  normalize: denom rows bounce via DRAM, reciprocal computed wide on [128,8],
  then partition-broadcast back with a stride-0 DRAM read; attn^T *= 1/denom.
  out partial = attn^T.T @ Wo_cols^T  (attn^T is directly the lhsT).
  x and Wq/Wk/Wv stream in as bf16 (halves input DMA); K^T is stored as two
  zero-padded per-head tiles so every scores matmul runs at full K=128.
  Measured: ~222 us HW exec (8 cores), rel err ~2e-3 vs fp32 reference.
"""
import sys

sys.path.insert(0, "/opt/trn_rl_repo")

import numpy as np
import ml_dtypes

import concourse.bass as bass
import concourse.bacc as bacc
import concourse.tile as tile
import concourse.mybir as mybir
from concourse.bass_utils import run_bass_kernel_spmd

B, S, D, H, HD = 2, 2048, 1024, 16, 64
HPC = 4            # heads per core
HDC = HPC * HD     # 256 hd dims per core
KC = D // 128      # 8 contraction chunks
TQ = S // 512      # 4 q-chunks of 512
TT = S // 128      # 16 token tiles of 128
SCALE = 1.0 / 8.0  # 1/sqrt(64)
NEG = -1.0e9

f32 = mybir.dt.float32
f32r = mybir.dt.float32r
bf16 = mybir.dt.bfloat16
PROJ_BF16 = True  # x and Wq/Wk/Wv in bf16 (halves input DMA); attention stays f32r
PROJ_DT = bf16 if PROJ_BF16 else f32r

_CACHE = {}


def _emit(tc, d, ctx):
    nc = tc.nc
    singles = ctx.enter_context(tc.tile_pool(name="singles", bufs=1))
    xt_pool = ctx.enter_context(tc.tile_pool(name="xt", bufs=6))
    probs_pool = ctx.enter_context(tc.tile_pool(name="probs", bufs=4))
    stage_pool = ctx.enter_context(tc.tile_pool(name="stage", bufs=3))
    norm_pool = ctx.enter_context(tc.tile_pool(name="norm", bufs=2))
    ps = ctx.enter_context(tc.tile_pool(name="ps", bufs=4, space="PSUM"))

    bias_sb = singles.tile([128, 6], f32)
    nc.sync.dma_start(out=bias_sb, in_=d["bias"][:])

    # ---- projections: Q^T, K^T, V^T  (K-outer, 8 psum cells = 4 ps2 tiles) ----
    qt_sb = singles.tile([128, 2, S], f32r, tag="qt")
    ktz_sb = singles.tile([128, 2, 2, S], f32r, tag="ktz")
    nc.vector.memset(ktz_sb[64:128, :, 0, :].bitcast(f32), 0.0)
    nc.vector.memset(ktz_sb[0:64, :, 1, :].bitcast(f32), 0.0)
    vtt_sb = singles.tile([128, 2, S], f32r, tag="vtt")
    w_sb = {}
    proj = [("xq", "wq", qt_sb, 0), ("xk", "wk", None, 2), ("xv", "wv", vtt_sb, 4)]
    for xnm, wnm, dst, bcol in proj:
        w_sb[wnm] = singles.tile([128, KC, HDC], PROJ_DT, tag=wnm, name=wnm)
        nc.sync.dma_start(
            out=w_sb[wnm], in_=d[wnm][:].rearrange("p (kc m) -> p kc m", kc=KC)
        )
        cellt = [
            ps.tile([128, 2, 512], f32, tag="ps2", name=f"cell_{xnm}_{i}")
            for i in range(4)
        ]

        def cell(mc, t):
            i = mc * TQ + t
            return cellt[i // 2][:, i % 2, :]

        for c in range(KC):
            xt = xt_pool.tile([128, S], PROJ_DT, tag="xt")
            nc.sync.dma_start(out=xt, in_=d[xnm][c * 128 : (c + 1) * 128, :])
            for mc in range(2):
                for t in range(TQ):
                    nc.tensor.matmul(
                        cell(mc, t),
                        w_sb[wnm][:, c, mc * 128 : (mc + 1) * 128],
                        xt[:, t * 512 : (t + 1) * 512],
                        start=(c == 0),
                        stop=(c == KC - 1),
                    )
        for mc in range(2):
            for t in range(TQ):
                tsl = slice(t * 512, (t + 1) * 512)
                if dst is None:  # K^T: split into zero-padded per-head tiles
                    nc.vector.tensor_scalar_add(
                        out=ktz_sb[0:64, mc, 0, tsl],
                        in0=cell(mc, t)[0:64, :],
                        scalar1=bias_sb[0:64, bcol + mc : bcol + mc + 1],
                    )
                    nc.vector.tensor_scalar_add(
                        out=ktz_sb[64:128, mc, 1, tsl],
                        in0=cell(mc, t)[64:128, :],
                        scalar1=bias_sb[64:128, bcol + mc : bcol + mc + 1],
                    )
                else:
                    nc.vector.tensor_scalar_add(
                        out=dst[:, mc, tsl],
                        in0=cell(mc, t),
                        scalar1=bias_sb[:, bcol + mc : bcol + mc + 1],
                    )

    # ---- V: transpose V^T -> V natural [tok, hd] per head, append ones ----
    ident = singles.tile([128, 128], f32r)
    nc.sync.dma_start(out=ident, in_=d["ident"][:])
    mask_sb = singles.tile([128, 4, 512], f32r)
    nc.sync.dma_start(out=mask_sb, in_=d["mask"][:])
    v_sb = [
        singles.tile([128, TT, 65], f32r, tag=f"v{h}", name=f"v{h}")
        for h in range(HPC)
    ]
    for h in range(HPC):
        nc.vector.memset(v_sb[h][:, :, 64:65].bitcast(f32), 1.0)
    for c in range(2):
        for t in range(TT):
            tp = ps.tile([128, 2, 512], f32r, tag="ps2")
            with nc.allow_low_precision(reason="f32r transpose; psum is fp32"):
                nc.tensor.transpose(
                    tp[:, 0, 0:128], vtt_sb[:, c, t * 128 : (t + 1) * 128], ident
                )
            for h2 in range(2):
                h = 2 * c + h2
                nc.vector.tensor_copy(
                    out=v_sb[h][:, t, 0:64], in_=tp[:, 0, h2 * 64 : (h2 + 1) * 64]
                )

    # ---- attention (per head-pair p, per q-chunk t, k-blocks in pairs) ----
    attnt_sb = singles.tile([128, 2, S], f32r, tag="attnt")
    for p in range(2):
        for t in range(TQ):
            q0 = t * 512
            qsl = slice(q0, q0 + 512)
            nkb = q0 // 128 + 4
            pvt = ps.tile([128, 2, 512], f32, tag="ps2", name=f"pv_{p}_{t}")
            for kb0 in range(0, nkb, 2):
                for h2 in range(2):
                    h = 2 * p + h2
                    scg = ps.tile([128, 2, 512], f32, tag="ps2", name="scg")
                    for j in range(2):
                        kb = kb0 + j
                        dg = kb - q0 // 128
                        nc.tensor.matmul(
                            scg[:, j, :],
                            ktz_sb[:, p, h2, kb * 128 : (kb + 1) * 128],
                            qt_sb[:, p, qsl],
                            start=True,
                            stop=(dg < 0),
                        )
                        if dg >= 0:
                            nc.tensor.matmul(
                                scg[:, j, :],
                                ident[:],
                                mask_sb[:, dg, :],
                                start=False,
                                stop=True,
                            )
                    pr = probs_pool.tile([128, 2, 512], f32r, tag="probs")
                    nc.scalar.activation(
                        out=pr[:],
                        in_=scg[:],
                        func=mybir.ActivationFunctionType.Exp,
                        scale=SCALE,
                    )
                    for j in range(2):
                        kb = kb0 + j
                        nc.tensor.matmul(
                            pvt[0:65, h2, :],
                            v_sb[h][:, kb, :],
                            pr[:, j, :],
                            start=(kb == 0),
                            stop=(kb == nkb - 1),
                        )
            dn = norm_pool.tile([65, 2, 512], f32, tag="dn")
            tmpb = norm_pool.tile([64, 512], f32r, tag="tmpb")
            bc_sb = norm_pool.tile([128, 512], f32, tag="bcs")
            # head A: rows 0-63 partition-preserving copy
            nc.vector.tensor_copy(out=attnt_sb[0:64, p, qsl], in_=pvt[0:64, 0, :])
            # head B: bounce through SBUF then DMA to partitions 64-127
            nc.vector.tensor_copy(out=tmpb[:], in_=pvt[0:64, 1, :])
            nc.sync.dma_start(out=attnt_sb[64:128, p, qsl], in_=tmpb[:])
            nc.vector.tensor_copy(out=dn[64:65, 0, :], in_=pvt[64:65, 0, :])
            nc.vector.tensor_copy(out=dn[64:65, 1, :], in_=pvt[64:65, 1, :])
            # denoms -> DRAM; read back wide [128,8]; exact reciprocal on
            # 128 lanes; back to DRAM; stride-0 broadcast read per head
            for h2 in range(2):
                nc.sync.dma_start(out=d["nscr"][p, t, h2, :], in_=dn[64:65, h2, :])
            wide = norm_pool.tile([128, 8], f32, tag="wide")
            wrec = norm_pool.tile([128, 8], f32, tag="wrec")
            flat_in = d["nscr"][p, t].rearrange("c q -> (c q)").rearrange(
                "(pp f) -> pp f", pp=128
            )
            nc.sync.dma_start(out=wide[:], in_=flat_in)
            with nc.allow_low_precision(reason="softmax denominators, fp32"):
                nc.vector.reciprocal(out=wrec[:], in_=wide[:])
            flat_out = d["nscr2"][p, t].rearrange("c q -> (c q)").rearrange(
                "(pp f) -> pp f", pp=128
            )
            nc.sync.dma_start(out=flat_out, in_=wrec[:])
            for h2 in range(2):
                srcd = d["nscr2"][p, t, h2, :]
                rep = bass.AP(
                    tensor=srcd.tensor,
                    offset=srcd.offset,
                    ap=[[0, 64]] + [list(e) for e in srcd.ap],
                )
                nc.sync.dma_start(
                    out=bc_sb[h2 * 64 : (h2 + 1) * 64, :], in_=rep
                )
            nc.vector.tensor_tensor(
                out=attnt_sb[:, p, qsl],
                in0=attnt_sb[:, p, qsl],
                in1=bc_sb[:],
                op=mybir.AluOpType.mult,
            )
            nc.vector.tensor_scalar_add(
                out=attnt_sb[:, p, qsl],
                in0=attnt_sb[:, p, qsl],
                scalar1=bias_sb[:, 4 + p : 5 + p],
            )

    # ---- output projection (partial over this core's 256 hd dims) ----
    wo_sb = singles.tile([128, 2, D], f32r)
    nc.sync.dma_start(out=wo_sb, in_=d["wo"][:].rearrange("p (c o) -> p c o", c=2))
    for t in range(TT):
        tsl = slice(t * 128, (t + 1) * 128)
        po = ps.tile([128, 2, 512], f32, tag="ps2", name=f"po_{t}")
        for o in range(2):
            osl = slice(o * 512, (o + 1) * 512)
            for c in range(2):
                nc.tensor.matmul(
                    po[:, o, :],
                    attnt_sb[:, c, tsl],
                    wo_sb[:, c, osl],
                    start=(c == 0),
                    stop=(c == 1),
                )
        st = stage_pool.tile([128, 2, 512], f32, tag="st")
        nc.vector.tensor_copy(out=st[:], in_=po[:])
        nc.sync.dma_start(out=d["out"][tsl, :], in_=st[:].rearrange("p a b -> p (a b)"))


def _build_nc():
    nc = bacc.Bacc()
    d = {}
    for nm in ("xq", "xk", "xv"):
        d[nm] = nc.declare_dram_parameter(nm, [D, S], PROJ_DT, isOutput=False)
    for nm in ("wq", "wk", "wv"):
        d[nm] = nc.declare_dram_parameter(nm, [128, KC * HDC], PROJ_DT, isOutput=False)
    d["wo"] = nc.declare_dram_parameter("wo", [128, 2 * D], f32r, isOutput=False)
    d["bias"] = nc.declare_dram_parameter("bias", [128, 6], f32, isOutput=False)
    d["mask"] = nc.declare_dram_parameter("mask", [128, 4, 512], f32r, isOutput=False)
    d["ident"] = nc.declare_dram_parameter("ident", [128, 128], f32r, isOutput=False)
    d["out"] = nc.declare_dram_parameter("out", [S, D], f32, isOutput=True)
    from contextlib import ExitStack

    d["nscr"] = nc.dram_tensor("nscr", [2, TQ, 2, 512], f32)
    d["nscr2"] = nc.dram_tensor("nscr2", [2, TQ, 2, 512], f32)
    with tile.TileContext(nc) as tc:
        with ExitStack() as ctx:
            _emit(tc, d, ctx)
    nc.compile()
    return nc


def _get_nc():
    if "nc" not in _CACHE:
        _CACHE["nc"] = _build_nc()
    return _CACHE["nc"]


_PROJ_NP = ml_dtypes.bfloat16 if PROJ_BF16 else np.float32


def _xarr(xt):
    return np.ascontiguousarray(xt).astype(_PROJ_NP)


def _warr(wt):  # [D, HDC] -> [128, KC*HDC] chunk-contiguous
    return np.ascontiguousarray(
        wt.reshape(KC, 128, HDC).transpose(1, 0, 2).reshape(128, KC * HDC)
    ).astype(_PROJ_NP)


def _woarr(wt):  # [HDC, D] -> [128, 2*D]
    return np.ascontiguousarray(
        wt.reshape(2, 128, D).transpose(1, 0, 2).reshape(128, 2 * D)
    )


def _host_consts():
    mask = np.zeros((128, 4, 512), np.float32)
    p = np.arange(128)[:, None]
    j = np.arange(512)[None, :]
    for dg in range(4):
        mask[:, dg, :] = np.where(128 * dg + p <= j, 0.0, NEG)
    ident = np.eye(128, dtype=np.float32)
    return mask, ident


def kernel(trace=False, **inputs):
    q = np.asarray(inputs["q"], np.float32)
    k = np.asarray(inputs["k"], np.float32)
    v = np.asarray(inputs["v"], np.float32)
    Wq = np.asarray(inputs["Wq"], np.float32)
    Wk = np.asarray(inputs["Wk"], np.float32)
    Wv = np.asarray(inputs["Wv"], np.float32)
    Wo = np.asarray(inputs["Wo"], np.float32)
    bq = np.asarray(inputs["bq"], np.float32)
    bk = np.asarray(inputs["bk"], np.float32)
    bv = np.asarray(inputs["bv"], np.float32)
    bo = np.asarray(inputs["bo"], np.float32)
    # inputs["mask"] is the causal tril mask, baked into the kernel.

    mask, ident = _host_consts()
    nc = _get_nc()
    in_maps = []
    for core in range(8):
        b, g = core // 4, core % 4
        sl = slice(g * HDC, (g + 1) * HDC)
        bias = np.zeros((128, 6), np.float32)
        for col, bvec in ((0, bq), (2, bk), (4, bv)):
            seg = bvec[sl].reshape(2, 128)
            bias[:, col] = seg[0]
            bias[:, col + 1] = seg[1]
        in_maps.append(
            {
                "xq": _xarr(q[b].T),
                "xk": _xarr(k[b].T),
                "xv": _xarr(v[b].T),
                "wq": _warr(Wq[sl, :].T),
                "wk": _warr(Wk[sl, :].T),
                "wv": _warr(Wv[sl, :].T),
                "wo": _woarr(Wo[:, sl].T),
                "bias": bias,
                "mask": mask,
                "ident": ident,
            }
        )
    res = run_bass_kernel_spmd(nc, in_maps, core_ids=list(range(8)), trace=trace)
    outs = [r["out"] for r in res.results]
    final = np.empty((B, S, D), np.float32)
    for b in range(B):
        final[b] = outs[4 * b] + outs[4 * b + 1] + outs[4 * b + 2] + outs[4 * b + 3]
        final[b] += bo
    if trace:
        kernel.last_exec_time_ns = res.exec_time_ns
        kernel.last_results = res
    return final
